# revision 32
# baseline (speedup 1.0000x reference)
"""Multi-head causal attention (B=4, T=1024, C=1024, H=16, D=64) on 8 TRN2 cores.

Sharding: tensor-parallel over heads. Core i owns heads {2i, 2i+1}:
  - x is replicated (sent pre-transposed as xT [C, B*T], bf16)
  - Wq/Wk/Wv sharded over heads -> per-core [C, 128] (2 heads concat on D)
  - row-parallel output projection: per-core Wp rows [128, C]; host sums the
    8 partial [B*T, C] outputs (the all-reduce) and adds bp.

Per-core kernel (bf16 matmuls, fp32 PSUM accumulation):
  for b in 0..3:
    qT/kT/vT [128(2 heads' d), 1024(t)] = W.T @ x[b].T       (PE, psum->sbuf)
    V_aug[h] [s,65] = transpose(vT) with a ones column        (PE transpose)
    per head h:
      scoresT [s_chunk=128, t] = K Q^T   (skip fully-causal-masked tiles,
                                          shrunken t-ranges on diagonal)
      expT = exp(scores/32)              (ACT, fused scale; no max-subtract:
                                          |scores|<~1 so exp is safe)
      diagonal 128-blocks *= upper-tri mask (DVE)
      outT_aug [65, t] += V_aug[h].T @ expT    (accumulate over s chunks;
                                          row 64 = softmax denominator)
      denom -> f32r ones-matmul partition-broadcast -> approx-reciprocal
      outT2[h*64:,:] = outT_aug[0:64] * rec2   (DVE, fused into psum copy)
    proj psum [t_tile 128, c 512] = outT2_tile.T @ Wp_l  -> sbuf -> DMA out
"""

import ml_dtypes
import numpy as np

B, T, C = 4, 1024, 1024
H, D = 16, 64
NCORES = 8
HPC = H // NCORES      # heads per core = 2
D2 = HPC * D           # 128
BT = B * T
SCALE = 1.0 / np.sqrt(np.float32(C))  # 1/32
BF16 = ml_dtypes.bfloat16

_compiled = None


def _split_multi_waits(nc, mybir, maxw=1):
    """Walrus in this container encodes at most one sync wait per
    instruction (fp32 self-loading matmuls and drains overflow).  Hoist
    excess waits onto same-engine NoOps inserted just before."""
    for fn in nc.m.functions:
        for bb in fn.blocks:
            new = []
            for inst in bb.instructions:
                si = inst.sync_info
                waits = list(si.on_wait) if (si is not None and si.on_wait) else []
                if len(waits) > maxw:
                    extra, keep = waits[:-maxw], waits[-maxw:]
                    for j, w in enumerate(extra):
                        new.append(
                            mybir.InstNoOp(
                                name=f"{inst.name}-wsplit{j}",
                                engine=inst.engine,
                                sync_info=mybir.SyncInfo(on_wait=[w], on_update=[]),
                                bass_nofuse=True,
                            )
                        )
                    inst.sync_info = mybir.SyncInfo(
                        on_wait=keep,
                        on_update=list(si.on_update) if si.on_update else [],
                    )
                new.append(inst)
            bb.instructions = new


_LDW_OPT = False


def _patch_ldw_opt():
    """Let walrus dedup back-to-back LDWEIGHTS of the same stationary."""
    import concourse.bass_utils as _bu

    if getattr(_bu, "_ldw_opt_patched", False):
        return
    _orig = _bu.run_command

    def _run(argv, **kw):
        if _LDW_OPT and isinstance(argv, list):
            argv = [
                "--enable-ldw-opt=true" if x == "--enable-ldw-opt=false" else x
                for x in argv
            ]
        return _orig(argv, **kw)

    _bu.run_command = _run
    _bu._ldw_opt_patched = True


def _build():
    import concourse.bass as bass
    import concourse.mybir as mybir
    import concourse.tile as tile

    _patch_ldw_opt()

    f32 = mybir.dt.float32
    f32r = mybir.dt.float32r
    bf = mybir.dt.bfloat16
    EXP = mybir.ActivationFunctionType.Exp

    nc = bass.Bass("TRN2", target_bir_lowering=False, debug=False, num_devices=NCORES)

    xT_d = nc.dram_tensor("xT", [C, BT], bf, kind="ExternalInput").ap()
    wq_d = nc.dram_tensor("wq", [C, D2], bf, kind="ExternalInput").ap()
    wk_d = nc.dram_tensor("wk", [C, D2], bf, kind="ExternalInput").ap()
    wv_d = nc.dram_tensor("wv", [C, D2], bf, kind="ExternalInput").ap()
    wp_d = nc.dram_tensor("wp", [D2, C], bf, kind="ExternalInput").ap()
    mask_d = nc.dram_tensor("mask", [128, 128], bf, kind="ExternalInput").ap()
    ident_d = nc.dram_tensor("ident", [128, 128], bf, kind="ExternalInput").ap()
    ones_d = nc.dram_tensor("ones", [128, 64], bf, kind="ExternalInput").ap()
    out_d = nc.dram_tensor("out", [BT, C], bf, kind="ExternalOutput").ap()

    KC = C // 128  # 8 contraction chunks over C
    NS = T // 128  # 8 s-chunks
    NH = 2         # two 512-wide t halves

    with tile.TileContext(nc) as tc:
        with (
            tc.tile_pool(name="const", bufs=1) as constp,
            tc.tile_pool(name="xin", bufs=3) as xinp,
            tc.tile_pool(name="qkv", bufs=3) as qkvp,
            tc.tile_pool(name="vaug", bufs=4) as vaugp,
            tc.tile_pool(name="exps", bufs=10) as expp,
            tc.tile_pool(name="smalls", bufs=4) as smallp,
            tc.tile_pool(name="outt", bufs=2) as outtp,
            tc.tile_pool(name="pout", bufs=4) as poutp,
            tc.tile_pool(name="dram", bufs=2, space="DRAM") as dramp,
            tc.tile_pool(name="ps512", bufs=4, space="PSUM") as ps512,
            tc.tile_pool(name="psatt", bufs=3, space="PSUM") as psatt,
            tc.tile_pool(name="psvt", bufs=1, space="PSUM") as psvt,
        ):
            # ---- constants ----
            wq_s = constp.tile([128, KC, D2], bf, tag="wq")
            wk_s = constp.tile([128, KC, D2], bf, tag="wk")
            wv_s = constp.tile([128, KC, D2], bf, tag="wv")
            wp_s = constp.tile([128, C], bf, tag="wp")
            mask_s = constp.tile([128, 128], bf, tag="mask")
            ident = constp.tile([128, 128], bf, tag="ident")
            nc.sync.dma_start(wq_s[:], wq_d.rearrange("(k p) m -> p k m", p=128))
            nc.sync.dma_start(wk_s[:], wk_d.rearrange("(k p) m -> p k m", p=128))
            nc.sync.dma_start(wv_s[:], wv_d.rearrange("(k p) m -> p k m", p=128))
            nc.sync.dma_start(wp_s[:], wp_d)
            nc.sync.dma_start(mask_s[:], mask_d)
            nc.sync.dma_start(ident[:], ident_d)

            def emit_qkv(b):
                # load x[b]^T as [c_part 128, kc 8, t 1024], one DMA per k
                xb = xinp.tile([128, KC, T], bf, tag="xb", name=f"xb{b}")
                for k in range(KC):
                    nc.sync.dma_start(
                        xb[:, k, :],
                        xT_d[k * 128:(k + 1) * 128, b * T:(b + 1) * T],
                    )
                qT = qkvp.tile([128, T], bf, tag="qT", name=f"qT{b}")
                kT = qkvp.tile([128, T], bf, tag="kT", name=f"kT{b}")
                vT = qkvp.tile([128, T], bf, tag="vT", name=f"vT{b}")
                for half in range(NH):
                    for w_s, oT in ((wq_s, qT), (wk_s, kT), (wv_s, vT)):
                        ps = ps512.tile([128, 512], f32, tag="ps512")
                        for k in range(KC):
                            nc.tensor.matmul(
                                ps[:],
                                w_s[:, k, :],
                                xb[:, k, half * 512:(half + 1) * 512],
                                start=(k == 0),
                                stop=(k == KC - 1),
                            )
                        nc.scalar.copy(oT[:, half * 512:(half + 1) * 512], ps[:])
                # V_aug per head: [s 128, 65] x NS chunks (col 64 = ones)
                vaug = [
                    vaugp.tile([128, NS, 65], bf, tag="vaug", name=f"vaug{b}_{h}")
                    for h in range(HPC)
                ]
                for h in range(HPC):
                    nc.sync.dma_start(
                        vaug[h][:, :, 64:65],
                        ones_d[:, 0:NS].rearrange("p (k o) -> p k o", o=1),
                    )
                for s in range(NS):
                    pv = psvt.tile([128, 128], bf, tag="psvt")
                    nc.tensor.transpose(
                        pv[:], vT[:, s * 128:(s + 1) * 128], ident[:]
                    )
                    for h in range(HPC):
                        nc.vector.tensor_copy(
                            vaug[h][:, s, 0:64], pv[:, h * 64:(h + 1) * 64]
                        )
                return qT, kT, vaug

            import concourse.bass as _bass

            def emit_normalize_half(b, half, po_h, outT2):
                t0 = half * 512
                den2 = smallp.tile(
                    [1, 2 * 512], f32, tag="den2", name=f"den2_{b}_{half}"
                )
                for h in range(HPC):
                    nc.scalar.copy(
                        den2[0:1, h * 512:(h + 1) * 512], po_h[h][64:65, 0:512]
                    )
                # Packed reciprocal: bounce the 1024 denominators through DRAM
                # to use all 128 DVE lanes (~0.1us instead of 2x3.3us), then
                # DMA-broadcast each [64, 512] operand back.
                scr_rec = dramp.tile(
                    [1, 1024], f32, tag="scr_rec", name=f"scrr_{b}_{half}"
                )
                packed = smallp.tile([128, 8], f32, tag="packed")
                nc.sync.dma_start(packed[:], den2[0:1, :])
                recp = smallp.tile([128, 8], f32, tag="recp")
                nc.vector.reciprocal(recp[:], packed[:])
                nc.sync.dma_start(
                    scr_rec[0, :].rearrange("(p f) -> p f", p=128), recp[:]
                )
                for h in range(HPC):
                    hp = slice(h * 64, (h + 1) * 64)
                    rec2 = smallp.tile(
                        [64, 512], f32, tag="rec2", name=f"rec2_{b}_{half}_{h}"
                    )
                    nc.sync.dma_start(
                        rec2[:],
                        _bass.AP(
                            scr_rec[:].tensor,
                            scr_rec[:].offset + 512 * h,
                            [[0, 64], [1, 512]],
                        ),
                    )
                    nc.vector.tensor_mul(
                        outT2[hp, t0:t0 + 512], po_h[h][0:64, 0:512], rec2[:]
                    )

            def emit_scores(b, qT, kT, vaug, outT2):
                """Full-width scores/exp per (h, s): one [128, 1024-s0] bf16
                matmul + exp; half0's attnV accumulates inline.  Returns the
                exp tiles and half0 psums; half1's attnV runs later against
                the same exp tiles (PE work that hides half0's normalize)."""
                exs = {}
                po0 = [
                    psatt.tile([128, 512], f32, tag="psatt", name=f"po0_{b}_{h}")
                    for h in range(HPC)
                ]
                for s in range(NS):
                    s0 = s * 128
                    d1 = max(0, s0 - 512)
                    for h in range(HPC):
                        hp = slice(h * 64, (h + 1) * 64)
                        ex = expp.tile(
                            [128, 1024], bf, tag="ex", bufs=18,
                            name=f"ex{b}_{h}_{s}"
                        )
                        exs[(h, s)] = ex
                        if s < 4:  # t-half0 piece: cols [s0, 512)
                            w0 = 512 - s0
                            pa = ps512.tile([128, 512], f32, tag="ps512")
                            nc.tensor.matmul(
                                pa[:, 0:w0],
                                kT[hp, s0:s0 + 128],
                                qT[hp, s0:512],
                                start=True,
                                stop=True,
                            )
                            nc.scalar.activation(
                                ex[:, 0:w0], pa[:, 0:w0], EXP, scale=float(SCALE)
                            )
                        # t-half1 piece: cols [max(512, s0), 1024)
                        w1 = 512 - d1
                        pb = ps512.tile([128, 512], f32, tag="ps512")
                        nc.tensor.matmul(
                            pb[:, 0:w1],
                            kT[hp, s0:s0 + 128],
                            qT[hp, 512 + d1:T],
                            start=True,
                            stop=True,
                        )
                        nc.scalar.activation(
                            ex[:, 512 - s0 + d1:T - s0],
                            pb[:, 0:w1],
                            EXP,
                            scale=float(SCALE),
                        )
                        nc.gpsimd.tensor_mul(
                            ex[:, 0:128], ex[:, 0:128], mask_s[:]
                        )
                    if s < 4:
                        for h in range(HPC):
                            nc.tensor.matmul(
                                po0[h][0:65, s0:512],
                                vaug[h][:, s, :],
                                exs[(h, s)][:, 0:512 - s0],
                                start=(s == 0),
                                stop=(s == 3),
                            )
                    if s == 4:
                        # attnV half0 is complete; start its normalize chain
                        # now so the recip DMA bounce overlaps scores s=4..7.
                        emit_normalize_half(b, 0, po0, outT2)
                return exs, po0

            def emit_attnv_half1(b, vaug, exs, mid=None):
                po1 = [
                    psatt.tile([128, 512], f32, tag="psatt", name=f"po1_{b}_{h}")
                    for h in range(HPC)
                ]
                for s in range(NS):
                    s0 = s * 128
                    d1 = max(0, s0 - 512)
                    for h in range(HPC):
                        nc.tensor.matmul(
                            po1[h][0:65, d1:512],
                            vaug[h][:, s, :],
                            exs[(h, s)][:, 512 - s0 + d1:T - s0],
                            start=(s == 0),
                            stop=(s == NS - 1),
                        )
                    if s == 3 and mid is not None:
                        mid()  # e.g. last b's proj half0 fills the PE stream
                return po1

            def emit_proj_half(b, outT2, half):
                # row-parallel projection: [t 128, c 512] tiles for this half's t
                for tt in range(half * 4, half * 4 + 4):
                    ob = poutp.tile([128, C], bf, tag="ob")
                    for ct in range(2):
                        pp = ps512.tile([128, 512], f32, tag="ps512")
                        nc.tensor.matmul(
                            pp[:],
                            outT2[:, tt * 128:(tt + 1) * 128],
                            wp_s[:, ct * 512:(ct + 1) * 512],
                            start=True,
                            stop=True,
                        )
                        if ct == 0:
                            nc.scalar.copy(ob[:, ct * 512:(ct + 1) * 512], pp[:])
                        else:
                            nc.vector.tensor_copy(
                                ob[:, ct * 512:(ct + 1) * 512], pp[:]
                            )
                    nc.sync.dma_start(
                        out_d[b * T + tt * 128:b * T + (tt + 1) * 128, :], ob[:]
                    )

            # Software pipeline: proj halves of b-1 are emitted between b's
            # stages so the in-order PE stream never waits on the normalize
            # chain (DVE/DMA) of the batch it just finished.
            prev = None
            for b in range(B):
                qT, kT, vaug = emit_qkv(b)
                outT2 = outtp.tile([128, T], bf, tag="outT2", name=f"outT2_{b}")
                if prev is not None:
                    emit_proj_half(prev[0], prev[1], 0)
                exs, po0 = emit_scores(b, qT, kT, vaug, outT2)
                if prev is not None:
                    emit_proj_half(prev[0], prev[1], 1)
                last = b == B - 1
                po1 = emit_attnv_half1(
                    b, vaug, exs,
                    mid=(lambda: emit_proj_half(b, outT2, 0)) if last else None,
                )
                emit_normalize_half(b, 1, po1, outT2)
                prev = (b, outT2)
            emit_proj_half(prev[0], prev[1], 1)

    _split_multi_waits(nc, mybir)
    return nc


def _get_compiled():
    global _compiled
    if _compiled is None:
        _compiled = _build()
    return _compiled


def _make_in_maps(x, Wq, Wk, Wv, Wp):
    xT = np.ascontiguousarray(
        np.asarray(x, dtype=np.float32).reshape(BT, C).T
    ).astype(BF16)  # [C, BT]
    mask = np.triu(np.ones((128, 128), dtype=BF16))  # keep j >= i
    ident = np.eye(128, dtype=BF16)
    ones = np.ones((128, 64), dtype=BF16)
    onesr = np.ones((1, 64), dtype=np.float32)
    in_maps = []
    for i in range(NCORES):
        h0 = i * HPC
        wq = np.ascontiguousarray(
            np.asarray(Wq[h0:h0 + HPC], dtype=np.float32).transpose(1, 0, 2).reshape(C, D2)
        ).astype(BF16)
        wk = np.ascontiguousarray(
            np.asarray(Wk[h0:h0 + HPC], dtype=np.float32).transpose(1, 0, 2).reshape(C, D2)
        ).astype(BF16)
        wv = np.ascontiguousarray(
            np.asarray(Wv[h0:h0 + HPC], dtype=np.float32).transpose(1, 0, 2).reshape(C, D2)
        ).astype(BF16)
        wp = np.ascontiguousarray(
            np.asarray(Wp, dtype=np.float32)[h0 * D:(h0 + HPC) * D, :]
        ).astype(BF16)
        in_maps.append(
            {"xT": xT, "wq": wq, "wk": wk, "wv": wv, "wp": wp, "mask": mask,
             "ident": ident, "ones": ones, "onesr": onesr}
        )
    return in_maps


def run(x, Wq, Wk, Wv, Wp, bp, trace=False, trace_cores=None):
    """Returns (full_output [B,T,C], BassKernelResults)."""
    from concourse.bass_utils import run_bass_kernel_spmd

    nc = _get_compiled()
    in_maps = _make_in_maps(x, Wq, Wk, Wv, Wp)
    kw = {}
    if trace:
        kw = {"trace": True, "trace_cores": trace_cores or [0]}
    res = run_bass_kernel_spmd(nc, in_maps, list(range(NCORES)), **kw)
    acc = np.zeros((BT, C), dtype=np.float32)
    for i in range(NCORES):
        acc += np.asarray(res.results[i]["out"], dtype=np.float32)
    acc += np.asarray(bp, dtype=np.float32)[None, :]
    return acc.reshape(B, T, C), res


def kernel(x, Wq, Wk, Wv, Wp, bp):
    out, _ = run(x, Wq, Wk, Wv, Wp, bp)
    return out


# revision 33
# speedup vs baseline: 1.0469x; 1.0469x over previous
"""Multi-head causal attention (B=4, T=1024, C=1024, H=16, D=64) on 8 TRN2 cores.

Sharding: tensor-parallel over heads. Core i owns heads {2i, 2i+1}:
  - x is replicated (sent pre-transposed as xT [C, B*T], bf16)
  - Wq/Wk/Wv sharded over heads -> per-core [C, 128] (2 heads concat on D)
  - row-parallel output projection: per-core Wp rows [128, C]; host sums the
    8 partial [B*T, C] outputs (the all-reduce) and adds bp.

Per-core kernel (bf16 matmuls, fp32 PSUM accumulation):
  for b in 0..3:
    qT/kT/vT [128(2 heads' d), 1024(t)] = W.T @ x[b].T       (PE, psum->sbuf)
    V_aug[h] [s,65] = transpose(vT) with a ones column        (PE transpose)
    per head h:
      scoresT [s_chunk=128, t] = K Q^T   (skip fully-causal-masked tiles,
                                          shrunken t-ranges on diagonal)
      expT = exp(scores/32)              (ACT, fused scale; no max-subtract:
                                          |scores|<~1 so exp is safe)
      diagonal 128-blocks *= upper-tri mask (DVE)
      outT_aug [65, t] += V_aug[h].T @ expT    (accumulate over s chunks;
                                          row 64 = softmax denominator)
      denom -> f32r ones-matmul partition-broadcast -> approx-reciprocal
      outT2[h*64:,:] = outT_aug[0:64] * rec2   (DVE, fused into psum copy)
    proj psum [t_tile 128, c 512] = outT2_tile.T @ Wp_l  -> sbuf -> DMA out
"""

import ml_dtypes
import numpy as np

B, T, C = 4, 1024, 1024
H, D = 16, 64
NCORES = 8
HPC = H // NCORES      # heads per core = 2
D2 = HPC * D           # 128
BT = B * T
SCALE = 1.0 / np.sqrt(np.float32(C))  # 1/32
BF16 = ml_dtypes.bfloat16

_compiled = None


def _split_multi_waits(nc, mybir, maxw=1):
    """Walrus in this container encodes at most one sync wait per
    instruction (fp32 self-loading matmuls and drains overflow).  Hoist
    excess waits onto same-engine NoOps inserted just before."""
    for fn in nc.m.functions:
        for bb in fn.blocks:
            new = []
            for inst in bb.instructions:
                si = inst.sync_info
                waits = list(si.on_wait) if (si is not None and si.on_wait) else []
                if len(waits) > maxw:
                    extra, keep = waits[:-maxw], waits[-maxw:]
                    for j, w in enumerate(extra):
                        new.append(
                            mybir.InstNoOp(
                                name=f"{inst.name}-wsplit{j}",
                                engine=inst.engine,
                                sync_info=mybir.SyncInfo(on_wait=[w], on_update=[]),
                                bass_nofuse=True,
                            )
                        )
                    inst.sync_info = mybir.SyncInfo(
                        on_wait=keep,
                        on_update=list(si.on_update) if si.on_update else [],
                    )
                new.append(inst)
            bb.instructions = new


_LDW_OPT = False


def _patch_ldw_opt():
    """Let walrus dedup back-to-back LDWEIGHTS of the same stationary."""
    import concourse.bass_utils as _bu

    if getattr(_bu, "_ldw_opt_patched", False):
        return
    _orig = _bu.run_command

    def _run(argv, **kw):
        if _LDW_OPT and isinstance(argv, list):
            argv = [
                "--enable-ldw-opt=true" if x == "--enable-ldw-opt=false" else x
                for x in argv
            ]
        return _orig(argv, **kw)

    _bu.run_command = _run
    _bu._ldw_opt_patched = True


def _build():
    import concourse.bass as bass
    import concourse.mybir as mybir
    import concourse.tile as tile

    _patch_ldw_opt()

    f32 = mybir.dt.float32
    f32r = mybir.dt.float32r
    bf = mybir.dt.bfloat16
    EXP = mybir.ActivationFunctionType.Exp

    nc = bass.Bass("TRN2", target_bir_lowering=False, debug=False, num_devices=NCORES)

    xT_d = nc.dram_tensor("xT", [C, BT], bf, kind="ExternalInput").ap()
    wq_d = nc.dram_tensor("wq", [C, D2], bf, kind="ExternalInput").ap()
    wk_d = nc.dram_tensor("wk", [C, D2], bf, kind="ExternalInput").ap()
    wv_d = nc.dram_tensor("wv", [C, D2], bf, kind="ExternalInput").ap()
    wp_d = nc.dram_tensor("wp", [D2, C], bf, kind="ExternalInput").ap()
    mask_d = nc.dram_tensor("mask", [128, 128], bf, kind="ExternalInput").ap()
    ident_d = nc.dram_tensor("ident", [128, 128], bf, kind="ExternalInput").ap()
    ones_d = nc.dram_tensor("ones", [128, 64], bf, kind="ExternalInput").ap()
    out_d = nc.dram_tensor("out", [BT, C], bf, kind="ExternalOutput").ap()

    KC = C // 128  # 8 contraction chunks over C
    NS = T // 128  # 8 s-chunks
    NH = 2         # two 512-wide t halves

    with tile.TileContext(nc) as tc:
        with (
            tc.tile_pool(name="const", bufs=1) as constp,
            tc.tile_pool(name="xin", bufs=3) as xinp,
            tc.tile_pool(name="qkv", bufs=3) as qkvp,
            tc.tile_pool(name="vaug", bufs=4) as vaugp,
            tc.tile_pool(name="exps", bufs=10) as expp,
            tc.tile_pool(name="smalls", bufs=4) as smallp,
            tc.tile_pool(name="outt", bufs=2) as outtp,
            tc.tile_pool(name="pout", bufs=4) as poutp,
            tc.tile_pool(name="dram", bufs=2, space="DRAM") as dramp,
            tc.tile_pool(name="ps512", bufs=4, space="PSUM") as ps512,
            tc.tile_pool(name="psatt", bufs=3, space="PSUM") as psatt,
            tc.tile_pool(name="psvt", bufs=1, space="PSUM") as psvt,
        ):
            # ---- constants ----
            wq_s = constp.tile([128, KC, D2], bf, tag="wq")
            wk_s = constp.tile([128, KC, D2], bf, tag="wk")
            wv_s = constp.tile([128, KC, D2], bf, tag="wv")
            wp_s = constp.tile([128, C], bf, tag="wp")
            mask_s = constp.tile([128, 128], bf, tag="mask")
            ident = constp.tile([128, 128], bf, tag="ident")
            nc.sync.dma_start(wq_s[:], wq_d.rearrange("(k p) m -> p k m", p=128))
            nc.sync.dma_start(wk_s[:], wk_d.rearrange("(k p) m -> p k m", p=128))
            nc.sync.dma_start(wv_s[:], wv_d.rearrange("(k p) m -> p k m", p=128))
            nc.sync.dma_start(wp_s[:], wp_d)
            nc.sync.dma_start(mask_s[:], mask_d)
            nc.sync.dma_start(ident[:], ident_d)

            def emit_qkv(b):
                # load x[b]^T as [c_part 128, kc 8, t 1024], one DMA per k
                xb = xinp.tile([128, KC, T], bf, tag="xb", name=f"xb{b}")
                for k in range(KC):
                    nc.sync.dma_start(
                        xb[:, k, :],
                        xT_d[k * 128:(k + 1) * 128, b * T:(b + 1) * T],
                    )
                qT = qkvp.tile([128, T], bf, tag="qT", name=f"qT{b}")
                kT = qkvp.tile([128, T], bf, tag="kT", name=f"kT{b}")
                vT = qkvp.tile([128, T], bf, tag="vT", name=f"vT{b}")
                for half in range(NH):
                    for w_s, oT in ((wq_s, qT), (wk_s, kT), (wv_s, vT)):
                        ps = ps512.tile([128, 512], f32, tag="ps512")
                        for k in range(KC):
                            nc.tensor.matmul(
                                ps[:],
                                w_s[:, k, :],
                                xb[:, k, half * 512:(half + 1) * 512],
                                start=(k == 0),
                                stop=(k == KC - 1),
                            )
                        nc.scalar.copy(oT[:, half * 512:(half + 1) * 512], ps[:])
                # V_aug per head: [s 128, 65] x NS chunks (col 64 = ones)
                vaug = [
                    vaugp.tile([128, NS, 65], bf, tag="vaug", name=f"vaug{b}_{h}")
                    for h in range(HPC)
                ]
                for h in range(HPC):
                    nc.sync.dma_start(
                        vaug[h][:, :, 64:65],
                        ones_d[:, 0:NS].rearrange("p (k o) -> p k o", o=1),
                    )
                for s in range(NS):
                    pv = psvt.tile([128, 128], bf, tag="psvt")
                    nc.tensor.transpose(
                        pv[:], vT[:, s * 128:(s + 1) * 128], ident[:]
                    )
                    for h in range(HPC):
                        nc.vector.tensor_copy(
                            vaug[h][:, s, 0:64], pv[:, h * 64:(h + 1) * 64]
                        )
                return qT, kT, vaug

            import concourse.bass as _bass

            def emit_normalize_half(b, half, po_h, outT2):
                t0 = half * 512
                den2 = smallp.tile(
                    [1, 2 * 512], f32, tag="den2", name=f"den2_{b}_{half}"
                )
                for h in range(HPC):
                    nc.vector.tensor_copy(
                        den2[0:1, h * 512:(h + 1) * 512], po_h[h][64:65, 0:512]
                    )
                # Packed reciprocal: bounce the 1024 denominators through DRAM
                # to use all 128 DVE lanes (~0.1us instead of 2x3.3us), then
                # DMA-broadcast each [64, 512] operand back.
                scr_rec = dramp.tile(
                    [1, 1024], f32, tag="scr_rec", name=f"scrr_{b}_{half}"
                )
                packed = smallp.tile([128, 8], f32, tag="packed")
                nc.sync.dma_start(packed[:], den2[0:1, :])
                recp = smallp.tile([128, 8], f32, tag="recp")
                nc.vector.reciprocal(recp[:], packed[:])
                nc.sync.dma_start(
                    scr_rec[0, :].rearrange("(p f) -> p f", p=128), recp[:]
                )
                for h in range(HPC):
                    hp = slice(h * 64, (h + 1) * 64)
                    rec2 = smallp.tile(
                        [64, 512], f32, tag="rec2", name=f"rec2_{b}_{half}_{h}"
                    )
                    nc.sync.dma_start(
                        rec2[:],
                        _bass.AP(
                            scr_rec[:].tensor,
                            scr_rec[:].offset + 512 * h,
                            [[0, 64], [1, 512]],
                        ),
                    )
                    nc.vector.tensor_mul(
                        outT2[hp, t0:t0 + 512], po_h[h][0:64, 0:512], rec2[:]
                    )

            def emit_scores(b, qT, kT, vaug, outT2):
                """Full-width scores/exp per (h, s): one [128, 1024-s0] bf16
                matmul + exp; half0's attnV accumulates inline.  Returns the
                exp tiles and half0 psums; half1's attnV runs later against
                the same exp tiles (PE work that hides half0's normalize)."""
                exs = {}
                po0 = [
                    psatt.tile([128, 512], f32, tag="psatt", name=f"po0_{b}_{h}")
                    for h in range(HPC)
                ]
                for s in range(NS):
                    s0 = s * 128
                    d1 = max(0, s0 - 512)
                    for h in range(HPC):
                        hp = slice(h * 64, (h + 1) * 64)
                        ex = expp.tile(
                            [128, 1024], bf, tag="ex", bufs=18,
                            name=f"ex{b}_{h}_{s}"
                        )
                        exs[(h, s)] = ex
                        if s < 4:  # t-half0 piece: cols [s0, 512)
                            w0 = 512 - s0
                            pa = ps512.tile([128, 512], f32, tag="ps512")
                            nc.tensor.matmul(
                                pa[:, 0:w0],
                                kT[hp, s0:s0 + 128],
                                qT[hp, s0:512],
                                start=True,
                                stop=True,
                            )
                            nc.scalar.activation(
                                ex[:, 0:w0], pa[:, 0:w0], EXP, scale=float(SCALE)
                            )
                        # t-half1 piece: cols [max(512, s0), 1024)
                        w1 = 512 - d1
                        pb = ps512.tile([128, 512], f32, tag="ps512")
                        nc.tensor.matmul(
                            pb[:, 0:w1],
                            kT[hp, s0:s0 + 128],
                            qT[hp, 512 + d1:T],
                            start=True,
                            stop=True,
                        )
                        nc.scalar.activation(
                            ex[:, 512 - s0 + d1:T - s0],
                            pb[:, 0:w1],
                            EXP,
                            scale=float(SCALE),
                        )
                        nc.gpsimd.tensor_mul(
                            ex[:, 0:128], ex[:, 0:128], mask_s[:]
                        )
                    if s < 4:
                        for h in range(HPC):
                            nc.tensor.matmul(
                                po0[h][0:65, s0:512],
                                vaug[h][:, s, :],
                                exs[(h, s)][:, 0:512 - s0],
                                start=(s == 0),
                                stop=(s == 3),
                            )
                    if s == 4:
                        # attnV half0 is complete; start its normalize chain
                        # now so the recip DMA bounce overlaps scores s=4..7.
                        emit_normalize_half(b, 0, po0, outT2)
                return exs, po0

            def emit_attnv_half1(b, vaug, exs, mid=None):
                po1 = [
                    psatt.tile([128, 512], f32, tag="psatt", name=f"po1_{b}_{h}")
                    for h in range(HPC)
                ]
                for s in range(NS):
                    s0 = s * 128
                    d1 = max(0, s0 - 512)
                    for h in range(HPC):
                        nc.tensor.matmul(
                            po1[h][0:65, d1:512],
                            vaug[h][:, s, :],
                            exs[(h, s)][:, 512 - s0 + d1:T - s0],
                            start=(s == 0),
                            stop=(s == NS - 1),
                        )
                    if s == 3 and mid is not None:
                        mid()  # e.g. last b's proj half0 fills the PE stream
                return po1

            def emit_proj_half(b, outT2, half):
                # row-parallel projection: [t 128, c 512] tiles for this half's t
                for tt in range(half * 4, half * 4 + 4):
                    ob = poutp.tile([128, C], bf, tag="ob")
                    for ct in range(2):
                        pp = ps512.tile([128, 512], f32, tag="ps512")
                        nc.tensor.matmul(
                            pp[:],
                            outT2[:, tt * 128:(tt + 1) * 128],
                            wp_s[:, ct * 512:(ct + 1) * 512],
                            start=True,
                            stop=True,
                        )
                        if ct == 0:
                            nc.scalar.copy(ob[:, ct * 512:(ct + 1) * 512], pp[:])
                        else:
                            nc.vector.tensor_copy(
                                ob[:, ct * 512:(ct + 1) * 512], pp[:]
                            )
                    nc.sync.dma_start(
                        out_d[b * T + tt * 128:b * T + (tt + 1) * 128, :], ob[:]
                    )

            # Software pipeline: proj halves of b-1 are emitted between b's
            # stages so the in-order PE stream never waits on the normalize
            # chain (DVE/DMA) of the batch it just finished.
            prev = None
            for b in range(B):
                qT, kT, vaug = emit_qkv(b)
                outT2 = outtp.tile([128, T], bf, tag="outT2", name=f"outT2_{b}")
                if prev is not None:
                    emit_proj_half(prev[0], prev[1], 0)
                exs, po0 = emit_scores(b, qT, kT, vaug, outT2)
                if prev is not None:
                    emit_proj_half(prev[0], prev[1], 1)
                last = b == B - 1
                po1 = emit_attnv_half1(
                    b, vaug, exs,
                    mid=(lambda: emit_proj_half(b, outT2, 0)) if last else None,
                )
                emit_normalize_half(b, 1, po1, outT2)
                prev = (b, outT2)
            emit_proj_half(prev[0], prev[1], 1)

    _split_multi_waits(nc, mybir)
    return nc


def _get_compiled():
    global _compiled
    if _compiled is None:
        _compiled = _build()
    return _compiled


def _make_in_maps(x, Wq, Wk, Wv, Wp):
    xT = np.ascontiguousarray(
        np.asarray(x, dtype=np.float32).reshape(BT, C).T
    ).astype(BF16)  # [C, BT]
    mask = np.triu(np.ones((128, 128), dtype=BF16))  # keep j >= i
    ident = np.eye(128, dtype=BF16)
    ones = np.ones((128, 64), dtype=BF16)
    onesr = np.ones((1, 64), dtype=np.float32)
    in_maps = []
    for i in range(NCORES):
        h0 = i * HPC
        wq = np.ascontiguousarray(
            np.asarray(Wq[h0:h0 + HPC], dtype=np.float32).transpose(1, 0, 2).reshape(C, D2)
        ).astype(BF16)
        wk = np.ascontiguousarray(
            np.asarray(Wk[h0:h0 + HPC], dtype=np.float32).transpose(1, 0, 2).reshape(C, D2)
        ).astype(BF16)
        wv = np.ascontiguousarray(
            np.asarray(Wv[h0:h0 + HPC], dtype=np.float32).transpose(1, 0, 2).reshape(C, D2)
        ).astype(BF16)
        wp = np.ascontiguousarray(
            np.asarray(Wp, dtype=np.float32)[h0 * D:(h0 + HPC) * D, :]
        ).astype(BF16)
        in_maps.append(
            {"xT": xT, "wq": wq, "wk": wk, "wv": wv, "wp": wp, "mask": mask,
             "ident": ident, "ones": ones, "onesr": onesr}
        )
    return in_maps


def run(x, Wq, Wk, Wv, Wp, bp, trace=False, trace_cores=None):
    """Returns (full_output [B,T,C], BassKernelResults)."""
    from concourse.bass_utils import run_bass_kernel_spmd

    nc = _get_compiled()
    in_maps = _make_in_maps(x, Wq, Wk, Wv, Wp)
    kw = {}
    if trace:
        kw = {"trace": True, "trace_cores": trace_cores or [0]}
    res = run_bass_kernel_spmd(nc, in_maps, list(range(NCORES)), **kw)
    acc = np.zeros((BT, C), dtype=np.float32)
    for i in range(NCORES):
        acc += np.asarray(res.results[i]["out"], dtype=np.float32)
    acc += np.asarray(bp, dtype=np.float32)[None, :]
    return acc.reshape(B, T, C), res


def kernel(x, Wq, Wk, Wv, Wp, bp):
    out, _ = run(x, Wq, Wk, Wv, Wp, bp)
    return out


# revision 35
# speedup vs baseline: 1.0726x; 1.0246x over previous
"""Multi-head causal attention (B=4, T=1024, C=1024, H=16, D=64) on 8 TRN2 cores.

Sharding: tensor-parallel over heads. Core i owns heads {2i, 2i+1}:
  - x is replicated (sent pre-transposed as xT [C, B*T], bf16)
  - Wq/Wk/Wv sharded over heads -> per-core [C, 128] (2 heads concat on D)
  - row-parallel output projection: per-core Wp rows [128, C]; host sums the
    8 partial [B*T, C] outputs (the all-reduce) and adds bp.

Per-core kernel (bf16 matmuls, fp32 PSUM accumulation):
  for b in 0..3:
    qT/kT/vT [128(2 heads' d), 1024(t)] = W.T @ x[b].T       (PE, psum->sbuf)
    V_aug[h] [s,65] = transpose(vT) with a ones column        (PE transpose)
    per head h:
      scoresT [s_chunk=128, t] = K Q^T   (skip fully-causal-masked tiles,
                                          shrunken t-ranges on diagonal)
      expT = exp(scores/32)              (ACT, fused scale; no max-subtract:
                                          |scores|<~1 so exp is safe)
      diagonal 128-blocks *= upper-tri mask (DVE)
      outT_aug [65, t] += V_aug[h].T @ expT    (accumulate over s chunks;
                                          row 64 = softmax denominator)
      denom -> f32r ones-matmul partition-broadcast -> approx-reciprocal
      outT2[h*64:,:] = outT_aug[0:64] * rec2   (DVE, fused into psum copy)
    proj psum [t_tile 128, c 512] = outT2_tile.T @ Wp_l  -> sbuf -> DMA out
"""

import ml_dtypes
import numpy as np

B, T, C = 4, 1024, 1024
H, D = 16, 64
NCORES = 8
HPC = H // NCORES      # heads per core = 2
D2 = HPC * D           # 128
BT = B * T
SCALE = 1.0 / np.sqrt(np.float32(C))  # 1/32
BF16 = ml_dtypes.bfloat16

_compiled = None


def _split_multi_waits(nc, mybir, maxw=1):
    """Walrus in this container encodes at most one sync wait per
    instruction (fp32 self-loading matmuls and drains overflow).  Hoist
    excess waits onto same-engine NoOps inserted just before."""
    for fn in nc.m.functions:
        for bb in fn.blocks:
            new = []
            for inst in bb.instructions:
                si = inst.sync_info
                waits = list(si.on_wait) if (si is not None and si.on_wait) else []
                if len(waits) > maxw:
                    extra, keep = waits[:-maxw], waits[-maxw:]
                    for j, w in enumerate(extra):
                        new.append(
                            mybir.InstNoOp(
                                name=f"{inst.name}-wsplit{j}",
                                engine=inst.engine,
                                sync_info=mybir.SyncInfo(on_wait=[w], on_update=[]),
                                bass_nofuse=True,
                            )
                        )
                    inst.sync_info = mybir.SyncInfo(
                        on_wait=keep,
                        on_update=list(si.on_update) if si.on_update else [],
                    )
                new.append(inst)
            bb.instructions = new


_LDW_OPT = False


def _patch_ldw_opt():
    """Let walrus dedup back-to-back LDWEIGHTS of the same stationary."""
    import concourse.bass_utils as _bu

    if getattr(_bu, "_ldw_opt_patched", False):
        return
    _orig = _bu.run_command

    def _run(argv, **kw):
        if _LDW_OPT and isinstance(argv, list):
            argv = [
                "--enable-ldw-opt=true" if x == "--enable-ldw-opt=false" else x
                for x in argv
            ]
        return _orig(argv, **kw)

    _bu.run_command = _run
    _bu._ldw_opt_patched = True


def _build():
    import concourse.bass as bass
    import concourse.mybir as mybir
    import concourse.tile as tile

    _patch_ldw_opt()

    f32 = mybir.dt.float32
    f32r = mybir.dt.float32r
    bf = mybir.dt.bfloat16
    EXP = mybir.ActivationFunctionType.Exp

    nc = bass.Bass("TRN2", target_bir_lowering=False, debug=False, num_devices=NCORES)

    xT_d = nc.dram_tensor("xT", [C, BT], bf, kind="ExternalInput").ap()
    wq_d = nc.dram_tensor("wq", [C, D2], bf, kind="ExternalInput").ap()
    wk_d = nc.dram_tensor("wk", [C, D2], bf, kind="ExternalInput").ap()
    wv_d = nc.dram_tensor("wv", [C, D2], bf, kind="ExternalInput").ap()
    wp_d = nc.dram_tensor("wp", [D2, C], bf, kind="ExternalInput").ap()
    mask_d = nc.dram_tensor("mask", [128, 128], bf, kind="ExternalInput").ap()
    ident_d = nc.dram_tensor("ident", [128, 128], bf, kind="ExternalInput").ap()
    ones_d = nc.dram_tensor("ones", [128, 64], bf, kind="ExternalInput").ap()
    out_d = nc.dram_tensor("out", [BT, C], bf, kind="ExternalOutput").ap()

    KC = C // 128  # 8 contraction chunks over C
    NS = T // 128  # 8 s-chunks
    NH = 2         # two 512-wide t halves

    with tile.TileContext(nc) as tc:
        with (
            tc.tile_pool(name="const", bufs=1) as constp,
            tc.tile_pool(name="xin", bufs=3) as xinp,
            tc.tile_pool(name="qkv", bufs=3) as qkvp,
            tc.tile_pool(name="vaug", bufs=4) as vaugp,
            tc.tile_pool(name="exps", bufs=10) as expp,
            tc.tile_pool(name="smalls", bufs=4) as smallp,
            tc.tile_pool(name="outt", bufs=2) as outtp,
            tc.tile_pool(name="pout", bufs=4) as poutp,
            tc.tile_pool(name="dram", bufs=2, space="DRAM") as dramp,
            tc.tile_pool(name="ps512", bufs=4, space="PSUM") as ps512,
            tc.tile_pool(name="psatt", bufs=3, space="PSUM") as psatt,
            tc.tile_pool(name="psvt", bufs=1, space="PSUM") as psvt,
        ):
            # ---- constants ----
            wq_s = constp.tile([128, KC, D2], bf, tag="wq")
            wk_s = constp.tile([128, KC, D2], bf, tag="wk")
            wv_s = constp.tile([128, KC, D2], bf, tag="wv")
            wp_s = constp.tile([128, C], bf, tag="wp")
            mask_s = constp.tile([128, 128], bf, tag="mask")
            ident = constp.tile([128, 128], bf, tag="ident")
            nc.sync.dma_start(wq_s[:], wq_d.rearrange("(k p) m -> p k m", p=128))
            nc.sync.dma_start(wk_s[:], wk_d.rearrange("(k p) m -> p k m", p=128))
            nc.sync.dma_start(wv_s[:], wv_d.rearrange("(k p) m -> p k m", p=128))

            def emit_qkv(b):
                # load x[b]^T as [c_part 128, kc 8, t 1024], one DMA per k
                xb = xinp.tile([128, KC, T], bf, tag="xb", name=f"xb{b}")
                for k in range(KC):
                    nc.sync.dma_start(
                        xb[:, k, :],
                        xT_d[k * 128:(k + 1) * 128, b * T:(b + 1) * T],
                    )
                if b == 0:
                    # non-critical constants: queued behind b0's x chunks
                    nc.sync.dma_start(mask_s[:], mask_d)
                    nc.sync.dma_start(ident[:], ident_d)
                    nc.sync.dma_start(wp_s[:], wp_d)
                qT = qkvp.tile([128, T], bf, tag="qT", name=f"qT{b}")
                kT = qkvp.tile([128, T], bf, tag="kT", name=f"kT{b}")
                vT = qkvp.tile([128, T], bf, tag="vT", name=f"vT{b}")
                for half in range(NH):
                    for w_s, oT in ((wq_s, qT), (wk_s, kT), (wv_s, vT)):
                        ps = ps512.tile([128, 512], f32, tag="ps512")
                        for k in range(KC):
                            nc.tensor.matmul(
                                ps[:],
                                w_s[:, k, :],
                                xb[:, k, half * 512:(half + 1) * 512],
                                start=(k == 0),
                                stop=(k == KC - 1),
                            )
                        nc.scalar.copy(oT[:, half * 512:(half + 1) * 512], ps[:])
                # V_aug per head: [s 128, 65] x NS chunks (col 64 = ones)
                vaug = [
                    vaugp.tile([128, NS, 65], bf, tag="vaug", name=f"vaug{b}_{h}")
                    for h in range(HPC)
                ]
                for h in range(HPC):
                    nc.sync.dma_start(
                        vaug[h][:, :, 64:65],
                        ones_d[:, 0:NS].rearrange("p (k o) -> p k o", o=1),
                    )
                for s in range(NS):
                    pv = psvt.tile([128, 128], bf, tag="psvt")
                    nc.tensor.transpose(
                        pv[:], vT[:, s * 128:(s + 1) * 128], ident[:]
                    )
                    for h in range(HPC):
                        nc.vector.tensor_copy(
                            vaug[h][:, s, 0:64], pv[:, h * 64:(h + 1) * 64]
                        )
                return qT, kT, vaug

            import concourse.bass as _bass

            def emit_normalize_half(b, half, po_h, outT2):
                t0 = half * 512
                den2 = smallp.tile(
                    [1, 2 * 512], f32, tag="den2", name=f"den2_{b}_{half}"
                )
                for h in range(HPC):
                    nc.vector.tensor_copy(
                        den2[0:1, h * 512:(h + 1) * 512], po_h[h][64:65, 0:512]
                    )
                # Packed reciprocal: bounce the 1024 denominators through DRAM
                # to use all 128 DVE lanes (~0.1us instead of 2x3.3us), then
                # DMA-broadcast each [64, 512] operand back.
                scr_rec = dramp.tile(
                    [1, 1024], f32, tag="scr_rec", name=f"scrr_{b}_{half}"
                )
                packed = smallp.tile([128, 8], f32, tag="packed")
                nc.sync.dma_start(packed[:], den2[0:1, :])
                recp = smallp.tile([128, 8], f32, tag="recp")
                nc.vector.reciprocal(recp[:], packed[:])
                nc.sync.dma_start(
                    scr_rec[0, :].rearrange("(p f) -> p f", p=128), recp[:]
                )
                for h in range(HPC):
                    hp = slice(h * 64, (h + 1) * 64)
                    rec2 = smallp.tile(
                        [64, 512], f32, tag="rec2", name=f"rec2_{b}_{half}_{h}"
                    )
                    nc.sync.dma_start(
                        rec2[:],
                        _bass.AP(
                            scr_rec[:].tensor,
                            scr_rec[:].offset + 512 * h,
                            [[0, 64], [1, 512]],
                        ),
                    )
                    nc.vector.tensor_mul(
                        outT2[hp, t0:t0 + 512], po_h[h][0:64, 0:512], rec2[:]
                    )

            def emit_scores(b, qT, kT, vaug, outT2):
                """Full-width scores/exp per (h, s): one [128, 1024-s0] bf16
                matmul + exp; half0's attnV accumulates inline.  Returns the
                exp tiles and half0 psums; half1's attnV runs later against
                the same exp tiles (PE work that hides half0's normalize)."""
                exs = {}
                po0 = [
                    psatt.tile([128, 512], f32, tag="psatt", name=f"po0_{b}_{h}")
                    for h in range(HPC)
                ]
                for s in range(NS):
                    s0 = s * 128
                    d1 = max(0, s0 - 512)
                    for h in range(HPC):
                        hp = slice(h * 64, (h + 1) * 64)
                        ex = expp.tile(
                            [128, 1024], bf, tag="ex", bufs=18,
                            name=f"ex{b}_{h}_{s}"
                        )
                        exs[(h, s)] = ex
                        if s < 4:  # t-half0 piece: cols [s0, 512)
                            w0 = 512 - s0
                            pa = ps512.tile([128, 512], f32, tag="ps512")
                            nc.tensor.matmul(
                                pa[:, 0:w0],
                                kT[hp, s0:s0 + 128],
                                qT[hp, s0:512],
                                start=True,
                                stop=True,
                            )
                            nc.scalar.activation(
                                ex[:, 0:w0], pa[:, 0:w0], EXP, scale=float(SCALE)
                            )
                        # t-half1 piece: cols [max(512, s0), 1024)
                        w1 = 512 - d1
                        pb = ps512.tile([128, 512], f32, tag="ps512")
                        nc.tensor.matmul(
                            pb[:, 0:w1],
                            kT[hp, s0:s0 + 128],
                            qT[hp, 512 + d1:T],
                            start=True,
                            stop=True,
                        )
                        nc.scalar.activation(
                            ex[:, 512 - s0 + d1:T - s0],
                            pb[:, 0:w1],
                            EXP,
                            scale=float(SCALE),
                        )
                        nc.gpsimd.tensor_mul(
                            ex[:, 0:128], ex[:, 0:128], mask_s[:]
                        )
                    if s < 4:
                        for h in range(HPC):
                            nc.tensor.matmul(
                                po0[h][0:65, s0:512],
                                vaug[h][:, s, :],
                                exs[(h, s)][:, 0:512 - s0],
                                start=(s == 0),
                                stop=(s == 3),
                            )
                    if s == 4:
                        # attnV half0 is complete; start its normalize chain
                        # now so the recip DMA bounce overlaps scores s=4..7.
                        emit_normalize_half(b, 0, po0, outT2)
                return exs, po0

            def emit_attnv_half1(b, vaug, exs, mid=None):
                po1 = [
                    psatt.tile([128, 512], f32, tag="psatt", name=f"po1_{b}_{h}")
                    for h in range(HPC)
                ]
                for s in range(NS):
                    s0 = s * 128
                    d1 = max(0, s0 - 512)
                    for h in range(HPC):
                        nc.tensor.matmul(
                            po1[h][0:65, d1:512],
                            vaug[h][:, s, :],
                            exs[(h, s)][:, 512 - s0 + d1:T - s0],
                            start=(s == 0),
                            stop=(s == NS - 1),
                        )
                    if s == 3 and mid is not None:
                        mid()  # e.g. last b's proj half0 fills the PE stream
                return po1

            def emit_proj_half(b, outT2, half):
                # row-parallel projection: [t 128, c 512] tiles for this half's t
                for tt in range(half * 4, half * 4 + 4):
                    ob = poutp.tile([128, C], bf, tag="ob")
                    for ct in range(2):
                        pp = ps512.tile([128, 512], f32, tag="ps512")
                        nc.tensor.matmul(
                            pp[:],
                            outT2[:, tt * 128:(tt + 1) * 128],
                            wp_s[:, ct * 512:(ct + 1) * 512],
                            start=True,
                            stop=True,
                        )
                        if ct == 0:
                            nc.scalar.copy(ob[:, ct * 512:(ct + 1) * 512], pp[:])
                        else:
                            nc.vector.tensor_copy(
                                ob[:, ct * 512:(ct + 1) * 512], pp[:]
                            )
                    nc.sync.dma_start(
                        out_d[b * T + tt * 128:b * T + (tt + 1) * 128, :], ob[:]
                    )

            # Software pipeline: proj halves of b-1 are emitted between b's
            # stages so the in-order PE stream never waits on the normalize
            # chain (DVE/DMA) of the batch it just finished.
            prev = None
            for b in range(B):
                qT, kT, vaug = emit_qkv(b)
                outT2 = outtp.tile([128, T], bf, tag="outT2", name=f"outT2_{b}")
                if prev is not None:
                    emit_proj_half(prev[0], prev[1], 0)
                exs, po0 = emit_scores(b, qT, kT, vaug, outT2)
                if prev is not None:
                    emit_proj_half(prev[0], prev[1], 1)
                last = b == B - 1
                po1 = emit_attnv_half1(
                    b, vaug, exs,
                    mid=(lambda: emit_proj_half(b, outT2, 0)) if last else None,
                )
                emit_normalize_half(b, 1, po1, outT2)
                prev = (b, outT2)
            emit_proj_half(prev[0], prev[1], 1)

    _split_multi_waits(nc, mybir)
    return nc


def _get_compiled():
    global _compiled
    if _compiled is None:
        _compiled = _build()
    return _compiled


def _make_in_maps(x, Wq, Wk, Wv, Wp):
    xT = np.ascontiguousarray(
        np.asarray(x, dtype=np.float32).reshape(BT, C).T
    ).astype(BF16)  # [C, BT]
    mask = np.triu(np.ones((128, 128), dtype=BF16))  # keep j >= i
    ident = np.eye(128, dtype=BF16)
    ones = np.ones((128, 64), dtype=BF16)
    onesr = np.ones((1, 64), dtype=np.float32)
    in_maps = []
    for i in range(NCORES):
        h0 = i * HPC
        wq = np.ascontiguousarray(
            np.asarray(Wq[h0:h0 + HPC], dtype=np.float32).transpose(1, 0, 2).reshape(C, D2)
        ).astype(BF16)
        wk = np.ascontiguousarray(
            np.asarray(Wk[h0:h0 + HPC], dtype=np.float32).transpose(1, 0, 2).reshape(C, D2)
        ).astype(BF16)
        wv = np.ascontiguousarray(
            np.asarray(Wv[h0:h0 + HPC], dtype=np.float32).transpose(1, 0, 2).reshape(C, D2)
        ).astype(BF16)
        wp = np.ascontiguousarray(
            np.asarray(Wp, dtype=np.float32)[h0 * D:(h0 + HPC) * D, :]
        ).astype(BF16)
        in_maps.append(
            {"xT": xT, "wq": wq, "wk": wk, "wv": wv, "wp": wp, "mask": mask,
             "ident": ident, "ones": ones, "onesr": onesr}
        )
    return in_maps


def run(x, Wq, Wk, Wv, Wp, bp, trace=False, trace_cores=None):
    """Returns (full_output [B,T,C], BassKernelResults)."""
    from concourse.bass_utils import run_bass_kernel_spmd

    nc = _get_compiled()
    in_maps = _make_in_maps(x, Wq, Wk, Wv, Wp)
    kw = {}
    if trace:
        kw = {"trace": True, "trace_cores": trace_cores or [0]}
    res = run_bass_kernel_spmd(nc, in_maps, list(range(NCORES)), **kw)
    acc = np.zeros((BT, C), dtype=np.float32)
    for i in range(NCORES):
        acc += np.asarray(res.results[i]["out"], dtype=np.float32)
    acc += np.asarray(bp, dtype=np.float32)[None, :]
    return acc.reshape(B, T, C), res


def kernel(x, Wq, Wk, Wv, Wp, bp):
    out, _ = run(x, Wq, Wk, Wv, Wp, bp)
    return out
